# revision 19
# baseline (speedup 1.0000x reference)
"""Liteformer fast attention kernel for Trainium2 (8 NeuronCores), v2.

Math (per (b,h) head, N=8192 tokens, C=K=E=64, m=256 anchors):
    xhat = qk / ||qk||_row
    phi  = tanh((xhat @ anchor.T) @ W_hash) = tanh(xhat @ G),  G = anchor.T @ W_hash  [64,64]
    kcum = phi.sum(axis=0)                                  [64]
    ctx  = phi.T @ v                                        [64,64]
    out  = (phi @ ctx + 65*v) / (phi @ kcum + 8192*65)[:,None]

Sharding: B*H = 32 heads split 4-per-core across 8 cores (fully independent).

v2 changes vs v1:
  - all inputs converted to fp16 on host; output fp16 on device, cast on host
  - s1T is ONE 128-contraction matmul against block-diag G2 (was 2x64)
  - kcum folded into tanh-T via activation accum_out (no fp32 kcum matmul)
  - sqrt batched per head (2 act-table swaps per head instead of ~10)
  - pass2: numer cols contiguous + dens at chunk end; 65*v added in PSUM by
    identity matmul; epilogue = 2 Act bias-copies + 1 DVE recip + 2 DVE mults
"""

import sys

sys.path.insert(0, "/opt/trn_rl_repo")

from contextlib import ExitStack

import numpy as np

import concourse.bass as bass
import concourse.tile as tile
from concourse import bacc, mybir
from concourse.bass_utils import run_bass_kernel_spmd
from concourse.masks import make_identity

B, H, N, C = 2, 16, 8192, 64
NBITS = 64
BIAS = NBITS + 1  # 65
DENOM_BIAS = float(N) * BIAS  # 532480
HEADS_PER_CORE = (B * H) // 8  # 4
NBLK = N // 1024  # 8 blocks of 1024 tokens per head
FP32 = mybir.dt.float32
FP16 = mybir.dt.float16
AX = mybir.AxisListType
AF = mybir.ActivationFunctionType
ALU = mybir.AluOpType


def bcast(ap, n):
    """Append a zero-stride broadcast dim of size n to an AP."""
    return bass.AP(tensor=ap.tensor, offset=ap.offset, ap=ap.ap + [[0, n]])


def build_head(tc, pools, consts, h, v_ap, out_h, qk_sb, rs, g_ps, norm_hook=None):
    nc = tc.nc
    temps, psX, psT, psN, psC, psO, persist = pools
    ident, ident65 = consts

    # g2 = block-diag [G 0; 0 G] fp16; lhsT for s1T, rhs for s1N
    g2 = temps.tile([128, 128], FP16, tag="g2")
    nc.gpsimd.memset(g2[:], 0.0)
    nc.scalar.copy(g2[0:64, 0:64], g_ps[:])
    nc.scalar.copy(g2[64:128, 64:128], g_ps[:])

    # whole-head v (1 DMA, 1KB/partition/blk chunks)
    vsb = persist.tile([128, NBLK, 8, 64], FP16, tag="vsb", bufs=2)
    nc.sync.dma_start(vsb[:], v_ap[h].rearrange("(blk p a) c -> p blk a c", p=128, a=8))

    phiT = persist.tile([128, NBLK, 512], FP16, tag="phiT", bufs=2)
    kcacc = temps.tile([128, NBLK], FP32, tag="kcacc", bufs=2)
    ctx_ps = psC.tile([64, 64], FP32, tag="ctx")

    out_blk = out_h.rearrange("(blk p a) c -> blk p (a c)", p=128, a=8)

    # ======================= PASS 1 ======================================
    if norm_hook is not None:
        norm_hook(0)  # next head's qk DMA, issued early for slack
    for blk in range(NBLK):
        # emit next head's norm phase in small steps spread across this head's
        # blocks so its sq/reduce chain doesn't stall the engine streams
        # (per-engine program order): steps 1-4 = sq+reduce chunks, 5 = sqrt+recip
        if norm_hook is not None and 2 <= blk <= 6:
            norm_hook(blk - 1)
        # xn = qk * rs (broadcast along c), fp16
        xn = temps.tile([128, 8, 64], FP16, tag="xn")
        nc.vector.tensor_mul(
            xn[:],
            qk_sb[:, blk],
            bcast(rs[:, blk * 8 : (blk + 1) * 8].rearrange("p (a o) -> p a o", o=1), 64),
        )

        # 4x PE transpose -> xt double-decker (fp16 psum), then to SBUF
        xt_ps = psX.tile([128, 512], FP16, tag="xt_ps")
        xn2 = xn[:].rearrange("p a c -> p (a c)")
        for j in range(4):
            nc.tensor.transpose(
                xt_ps[:, j * 128 : (j + 1) * 128],
                xn2[:, j * 128 : (j + 1) * 128],
                ident[:],
            )
        xt = temps.tile([128, 512], FP16, tag="xt")
        nc.vector.tensor_copy(xt[:], xt_ps[:])

        # s1T: ONE matmul, block-diag g2 as lhsT -> pT [128(2 decks k), 512 tok]
        pT_ps = psT.tile([128, 512], FP32, tag="pT")
        nc.tensor.matmul(pT_ps[:], g2[:], xt[:], start=True, stop=True)
        # tanh + fold kcum partial (sum over tokens) into accum_out
        nc.scalar.activation(
            phiT[:, blk, :], pT_ps[:], AF.Tanh, accum_out=kcacc[:, blk : blk + 1]
        )

        # s1N: 4 matmuls, rhs = [G|G] block-diag -> phi token-major
        pN_ps = psN.tile([128, 512], FP32, tag="pN")
        for j in range(4):
            nc.tensor.matmul(
                pN_ps[:, j * 128 : (j + 1) * 128],
                xt[:, j * 128 : (j + 1) * 128],
                g2[:],
                start=True,
                stop=True,
            )
        phi = temps.tile([128, 8, 64], FP16, tag="phi")
        nc.scalar.activation(phi[:].rearrange("p a c -> p (a c)"), pN_ps[:], AF.Tanh)

        # ctx += phi_a.T @ v_a
        for a in range(8):
            nc.tensor.matmul(
                ctx_ps[:],
                phi[:, a, :],
                vsb[:, blk, a, :],
                start=(blk == 0 and a == 0),
                stop=(blk == NBLK - 1 and a == 7),
            )

    # ---- kcum: [128,NBLK] -> [128,1] -> fold decks -> [64,1] -------------
    kcr = temps.tile([128, 1], FP32, tag="kcr")
    nc.vector.reduce_sum(kcr[:], kcacc[:], axis=AX.X)
    # fold decks: kcs[k] = kcr[k] + kcr[64+k] (stage upper half to base 0 first,
    # DVE tensor_tensor requires equal base partitions for SB inputs)
    kup = temps.tile([64, 1], FP32, tag="kup")
    nc.scalar.copy(kup[:], kcr[64:128, :])
    kcs = temps.tile([64, 1], FP32, tag="kcs")
    nc.vector.tensor_tensor(kcs[:], kcr[0:64, :], kup[:], op=ALU.add)

    # cc_n: [128,130] fp16 = [[ctx,0,kcs,0],[0,ctx,0,kcs]] (numer cols 0:128, dens 128:130)
    cc_n = temps.tile([128, 130], FP16, tag="cc_n")
    nc.gpsimd.memset(cc_n[:], 0.0)
    nc.scalar.copy(cc_n[0:64, 0:64], ctx_ps[:])
    nc.scalar.copy(cc_n[64:128, 64:128], ctx_ps[:])
    nc.scalar.copy(cc_n[0:64, 128:129], kcs[:])
    nc.scalar.copy(cc_n[64:128, 129:130], kcs[:])

    # ======================= PASS 2 ======================================
    for blk in range(NBLK):
        out_sb = temps.tile([128, 512], FP16, tag="out_sb", bufs=2)
        dnb = temps.tile([128, 8], FP32, tag="dnb", bufs=2)
        o_tiles = []
        for half in range(2):
            o = psO.tile([128, 260], FP32, tag="o", bufs=2)
            o_tiles.append(o)
            for q in range(2):  # chunk within half
                ch = half * 2 + q
                nc.tensor.matmul(
                    o[:, q * 130 : q * 130 + 130],
                    phiT[:, blk, ch * 128 : (ch + 1) * 128],
                    cc_n[:],
                    start=True,
                    stop=False,
                    skip_group_check=True,
                )
                nc.tensor.matmul(
                    o[:, q * 130 : q * 130 + 128],
                    ident65[:],
                    vsb[:, blk, ch * 2 : ch * 2 + 2, :],
                    start=False,
                    stop=True,
                    skip_group_check=True,
                )
            t = o[:]
            den = bass.AP(tensor=t.tensor, offset=t.offset + 128,
                          ap=[t.ap[0], [130, 2], [1, 2]])
            # dnb = den + N*BIAS   (Act Copy-with-bias; same table as tanh)
            nc.scalar.activation(
                dnb[:, half * 4 : half * 4 + 4].rearrange("p (c q) -> p c q", q=2),
                den, AF.Copy, bias=DENOM_BIAS,
            )
        rec = temps.tile([128, 8], FP32, tag="rec", bufs=2)
        nc.vector.reciprocal(rec[:], dnb[:])
        for half in range(2):
            t = o_tiles[half][:]
            numer = bass.AP(tensor=t.tensor, offset=t.offset,
                            ap=[t.ap[0], [130, 2], [64, 2], [1, 64]])
            rc = rec[:, half * 4 : half * 4 + 4].rearrange("p (c q) -> p c q", q=2)
            nc.vector.tensor_tensor(
                out_sb[:, half * 256 : half * 256 + 256].rearrange(
                    "p (c q e) -> p c q e", q=2, e=64
                ),
                numer,
                bcast(rc, 64),
                op=ALU.mult,
            )
        nc.sync.dma_start(out_blk[blk], out_sb[:])


def build_core(tc, pools, consts, qk_ap, v_ap, a_ap, w_ap, out_ap):
    nc = tc.nc
    temps, psX, psT, psN, psC, psO, persist = pools

    qk_tiles = [None] * HEADS_PER_CORE
    rs_tiles = [None] * HEADS_PER_CORE
    norm_state = {}

    def norm_step(h, step):
        # step 0: whole-head qk DMA; 1-4: sq+reduce over 2-block chunks;
        # 5: sqrt + recip + fp16 convert
        if step == 0:
            qk_sb = persist.tile(
                [128, NBLK, 8, 64], FP16, tag=f"qk{h}", bufs=1, name=f"qk_sb{h}"
            )
            nc.sync.dma_start(
                qk_sb[:], qk_ap[h].rearrange("(blk p a) c -> p blk a c", p=128, a=8)
            )
            nsq = temps.tile([128, 64], FP32, tag=f"nsq{h}", bufs=1, name=f"nsq{h}")
            qk_tiles[h] = qk_sb
            norm_state[h] = nsq
        elif step <= 4:
            j = step - 1
            qk_sb, nsq = qk_tiles[h], norm_state[h]
            sq = temps.tile([128, 2, 8, 64], FP16, tag="sq", name="sq")
            nc.vector.tensor_mul(
                sq[:], qk_sb[:, 2 * j : 2 * j + 2], qk_sb[:, 2 * j : 2 * j + 2]
            )
            nc.vector.reduce_sum(
                nsq[:, 16 * j : 16 * j + 16],
                sq[:].rearrange("p b a c -> p (b a) c"),
                axis=AX.X,
            )
        else:
            nsq = norm_state[h]
            nrm = temps.tile([128, 64], FP32, tag=f"nrm{h}", bufs=1, name=f"nrm{h}")
            nc.scalar.sqrt(nrm[:], nsq[:])
            rs = temps.tile([128, 64], FP32, tag=f"rs{h}", bufs=1, name=f"rs{h}")
            nc.vector.reciprocal(rs[:], nrm[:])
            rs16 = temps.tile([128, 64], FP16, tag=f"rs16{h}", bufs=1, name=f"rs16{h}")
            nc.scalar.copy(rs16[:], rs[:])
            rs_tiles[h] = rs16

    for s in range(6):
        norm_step(0, s)
    for h in range(HEADS_PER_CORE):
        hook = (
            (lambda step, hh=h + 1: norm_step(hh, step))
            if h + 1 < HEADS_PER_CORE
            else None
        )
        # G = anchor.T @ W_hash for this head
        a_sb = temps.tile([128, 2, 64], FP16, tag="a_sb")
        w_sb = temps.tile([128, 2, 64], FP16, tag="w_sb")
        nc.sync.dma_start(a_sb[:], a_ap[h].rearrange("(t p) c -> p t c", p=128))
        nc.sync.dma_start(w_sb[:], w_ap[h].rearrange("(t p) c -> p t c", p=128))
        g_ps = psT.tile([64, 64], FP32, tag="pT")
        for t in range(2):
            nc.tensor.matmul(
                g_ps[:], a_sb[:, t, :], w_sb[:, t, :], start=(t == 0), stop=(t == 1)
            )
        build_head(
            tc, pools, consts, h, v_ap, out_ap[h], qk_tiles[h], rs_tiles[h], g_ps,
            norm_hook=hook,
        )


def build_bass(repeat=1):
    nc = bacc.Bacc("TRN2", target_bir_lowering=False, debug=False, num_devices=8)
    hp = HEADS_PER_CORE
    qk_ap = nc.dram_tensor("qk", (hp, N, C), FP16, kind="ExternalInput").ap()
    v_ap = nc.dram_tensor("v", (hp, N, C), FP16, kind="ExternalInput").ap()
    a_ap = nc.dram_tensor("anchor", (hp, 256, C), FP16, kind="ExternalInput").ap()
    w_ap = nc.dram_tensor("W_hash", (hp, 256, NBITS), FP16, kind="ExternalInput").ap()
    out_ap = nc.dram_tensor("out", (hp, N, C), FP16, kind="ExternalOutput").ap()

    with tile.TileContext(nc) as tc:
        with ExitStack() as ctx:
            singles = ctx.enter_context(tc.tile_pool(name="singles", bufs=1))
            temps = ctx.enter_context(tc.tile_pool(name="temps", bufs=3))
            persist = ctx.enter_context(tc.tile_pool(name="persist", bufs=1))
            psX = ctx.enter_context(tc.tile_pool(name="psX", bufs=1, space="PSUM"))
            psT = ctx.enter_context(tc.tile_pool(name="psT", bufs=2, space="PSUM"))
            psN = ctx.enter_context(tc.tile_pool(name="psN", bufs=2, space="PSUM"))
            psC = ctx.enter_context(tc.tile_pool(name="psC", bufs=1, space="PSUM"))
            psO = ctx.enter_context(tc.tile_pool(name="psO", bufs=2, space="PSUM"))
            pools = (temps, psX, psT, psN, psC, psO, persist)

            ident = singles.tile([128, 128], FP16)
            make_identity(nc, ident[:])
            ident65 = singles.tile([128, 128], FP16)
            nc.scalar.activation(ident65[:], ident[:], AF.Copy, scale=float(BIAS))
            consts = (ident, ident65)

            if repeat == 1:
                build_core(tc, pools, consts, qk_ap, v_ap, a_ap, w_ap, out_ap)
            else:
                with tc.For_i(0, repeat, 1):
                    build_core(tc, pools, consts, qk_ap, v_ap, a_ap, w_ap, out_ap)
    nc.compile()
    return nc


_NC_CACHE = None
_RUN_KWARGS = {}
_LAST_RESULTS = None


def kernel(qk, v, anchor, W_hash):
    global _NC_CACHE
    if _NC_CACHE is None:
        _NC_CACHE = build_bass()
    nc = _NC_CACHE

    qk = np.ascontiguousarray(qk, dtype=np.float16).reshape(B * H, N, C)
    v = np.ascontiguousarray(v, dtype=np.float16).reshape(B * H, N, C)
    anchor = np.ascontiguousarray(anchor, dtype=np.float16)
    W_hash = np.ascontiguousarray(W_hash, dtype=np.float16)

    in_maps = []
    for core in range(8):
        bh = np.arange(core * HEADS_PER_CORE, (core + 1) * HEADS_PER_CORE)
        h_idx = bh % H
        in_maps.append(
            {
                "qk": qk[bh],
                "v": v[bh],
                "anchor": np.ascontiguousarray(anchor[h_idx]),
                "W_hash": np.ascontiguousarray(W_hash[h_idx]),
            }
        )

    res = run_bass_kernel_spmd(nc, in_maps, core_ids=list(range(8)), **_RUN_KWARGS)
    global _LAST_RESULTS
    _LAST_RESULTS = res
    out = np.concatenate([res.results[c]["out"] for c in range(8)], axis=0)
    return out.reshape(B, H, N, C).astype(np.float32)


# revision 20
# speedup vs baseline: 1.0455x; 1.0455x over previous
"""Liteformer fast attention kernel for Trainium2 (8 NeuronCores), v2.

Math (per (b,h) head, N=8192 tokens, C=K=E=64, m=256 anchors):
    xhat = qk / ||qk||_row
    phi  = tanh((xhat @ anchor.T) @ W_hash) = tanh(xhat @ G),  G = anchor.T @ W_hash  [64,64]
    kcum = phi.sum(axis=0)                                  [64]
    ctx  = phi.T @ v                                        [64,64]
    out  = (phi @ ctx + 65*v) / (phi @ kcum + 8192*65)[:,None]

Sharding: B*H = 32 heads split 4-per-core across 8 cores (fully independent).

v2 changes vs v1:
  - all inputs converted to fp16 on host; output fp16 on device, cast on host
  - s1T is ONE 128-contraction matmul against block-diag G2 (was 2x64)
  - kcum folded into tanh-T via activation accum_out (no fp32 kcum matmul)
  - sqrt batched per head (2 act-table swaps per head instead of ~10)
  - pass2: numer cols contiguous + dens at chunk end; 65*v added in PSUM by
    identity matmul; epilogue = 2 Act bias-copies + 1 DVE recip + 2 DVE mults
"""

import sys

sys.path.insert(0, "/opt/trn_rl_repo")

from contextlib import ExitStack

import numpy as np

import concourse.bass as bass
import concourse.tile as tile
from concourse import bacc, mybir
from concourse.bass_utils import run_bass_kernel_spmd
from concourse.masks import make_identity

B, H, N, C = 2, 16, 8192, 64
NBITS = 64
BIAS = NBITS + 1  # 65
DENOM_BIAS = float(N) * BIAS  # 532480
HEADS_PER_CORE = (B * H) // 8  # 4
NBLK = N // 1024  # 8 blocks of 1024 tokens per head
FP32 = mybir.dt.float32
FP16 = mybir.dt.float16
AX = mybir.AxisListType
AF = mybir.ActivationFunctionType
ALU = mybir.AluOpType


def bcast(ap, n):
    """Append a zero-stride broadcast dim of size n to an AP."""
    return bass.AP(tensor=ap.tensor, offset=ap.offset, ap=ap.ap + [[0, n]])


def build_head(tc, pools, consts, h, v_ap, out_h, qk_sb, rs, g_ps, norm_hook=None):
    nc = tc.nc
    temps, psX, psT, psN, psC, psO, persist = pools
    ident, ident65 = consts

    # g2 = block-diag [G 0; 0 G] fp16; lhsT for s1T, rhs for s1N
    g2 = temps.tile([128, 128], FP16, tag="g2")
    nc.gpsimd.memset(g2[:], 0.0)
    nc.scalar.copy(g2[0:64, 0:64], g_ps[:])
    nc.scalar.copy(g2[64:128, 64:128], g_ps[:])

    # whole-head v (1 DMA, 1KB/partition/blk chunks)
    vsb = persist.tile([128, NBLK, 8, 64], FP16, tag="vsb", bufs=2)
    nc.sync.dma_start(vsb[:], v_ap[h].rearrange("(blk p a) c -> p blk a c", p=128, a=8))

    phiT = persist.tile([128, NBLK, 512], FP16, tag="phiT", bufs=2)
    kcacc = temps.tile([128, NBLK], FP32, tag="kcacc", bufs=2)
    ctx_ps = psC.tile([64, 64], FP32, tag="ctx")

    out_blk = out_h.rearrange("(blk p a) c -> blk p (a c)", p=128, a=8)

    # ======================= PASS 1 ======================================
    if norm_hook is not None:
        norm_hook(0)  # next head's qk DMA, issued early for slack
    for blk in range(NBLK):
        # emit next head's norm phase in small steps spread across this head's
        # blocks so its sq/reduce chain doesn't stall the engine streams
        # (per-engine program order): steps 1-4 = sq+reduce chunks, 5 = sqrt+recip
        if norm_hook is not None and 2 <= blk <= 6:
            norm_hook(blk - 1)
        # xn = qk * rs (broadcast along c), fp16
        xn = temps.tile([128, 8, 64], FP16, tag="xn")
        nc.vector.tensor_mul(
            xn[:],
            qk_sb[:, blk],
            bcast(rs[:, blk * 8 : (blk + 1) * 8].rearrange("p (a o) -> p a o", o=1), 64),
        )

        # 4x PE transpose -> xt double-decker (fp16 psum), then to SBUF
        xt_ps = psX.tile([128, 512], FP16, tag="xt_ps")
        xn2 = xn[:].rearrange("p a c -> p (a c)")
        for j in range(4):
            nc.tensor.transpose(
                xt_ps[:, j * 128 : (j + 1) * 128],
                xn2[:, j * 128 : (j + 1) * 128],
                ident[:],
            )
        xt = temps.tile([128, 512], FP16, tag="xt")
        nc.vector.tensor_copy(xt[:], xt_ps[:])

        # s1T: ONE matmul, block-diag g2 as lhsT -> pT [128(2 decks k), 512 tok]
        pT_ps = psT.tile([128, 512], FP32, tag="pT")
        nc.tensor.matmul(pT_ps[:], g2[:], xt[:], start=True, stop=True)
        # tanh + fold kcum partial (sum over tokens) into accum_out
        nc.scalar.activation(
            phiT[:, blk, :], pT_ps[:], AF.Tanh, accum_out=kcacc[:, blk : blk + 1]
        )

        # s1N: 4 matmuls, rhs = [G|G] block-diag -> phi token-major
        pN_ps = psN.tile([128, 512], FP32, tag="pN")
        for j in range(4):
            nc.tensor.matmul(
                pN_ps[:, j * 128 : (j + 1) * 128],
                xt[:, j * 128 : (j + 1) * 128],
                g2[:],
                start=True,
                stop=True,
            )
        phi = temps.tile([128, 8, 64], FP16, tag="phi")
        nc.scalar.activation(phi[:].rearrange("p a c -> p (a c)"), pN_ps[:], AF.Tanh)

        # ctx += phi_a.T @ v_a
        for a in range(8):
            nc.tensor.matmul(
                ctx_ps[:],
                phi[:, a, :],
                vsb[:, blk, a, :],
                start=(blk == 0 and a == 0),
                stop=(blk == NBLK - 1 and a == 7),
            )

    # ---- kcum: [128,NBLK] -> [128,1] -> fold decks -> [64,1] -------------
    kcr = temps.tile([128, 1], FP32, tag="kcr")
    nc.vector.reduce_sum(kcr[:], kcacc[:], axis=AX.X)
    # fold decks: kcs[k] = kcr[k] + kcr[64+k] (stage upper half to base 0 first,
    # DVE tensor_tensor requires equal base partitions for SB inputs)
    kup = temps.tile([64, 1], FP32, tag="kup")
    nc.scalar.copy(kup[:], kcr[64:128, :])
    kcs = temps.tile([64, 1], FP32, tag="kcs")
    nc.vector.tensor_tensor(kcs[:], kcr[0:64, :], kup[:], op=ALU.add)

    # cc_n: [128,130] fp16 = [[ctx,0,kcs,0],[0,ctx,0,kcs]] (numer cols 0:128, dens 128:130)
    cc_n = temps.tile([128, 130], FP16, tag="cc_n")
    nc.gpsimd.memset(cc_n[:], 0.0)
    nc.scalar.copy(cc_n[0:64, 0:64], ctx_ps[:])
    nc.scalar.copy(cc_n[64:128, 64:128], ctx_ps[:])
    nc.scalar.copy(cc_n[0:64, 128:129], kcs[:])
    nc.scalar.copy(cc_n[64:128, 129:130], kcs[:])

    # ======================= PASS 2 ======================================
    for blk in range(NBLK):
        out_sb = temps.tile([128, 512], FP16, tag="out_sb", bufs=2)
        dnb = temps.tile([128, 8], FP32, tag="dnb", bufs=2)
        o_tiles = []
        for half in range(2):
            o = psO.tile([128, 260], FP32, tag="o", bufs=2)
            o_tiles.append(o)
            for q in range(2):  # chunk within half
                ch = half * 2 + q
                nc.tensor.matmul(
                    o[:, q * 130 : q * 130 + 130],
                    phiT[:, blk, ch * 128 : (ch + 1) * 128],
                    cc_n[:],
                    start=True,
                    stop=False,
                    skip_group_check=True,
                )
                nc.tensor.matmul(
                    o[:, q * 130 : q * 130 + 128],
                    ident65[:],
                    vsb[:, blk, ch * 2 : ch * 2 + 2, :],
                    start=False,
                    stop=True,
                    skip_group_check=True,
                )
            t = o[:]
            den = bass.AP(tensor=t.tensor, offset=t.offset + 128,
                          ap=[t.ap[0], [130, 2], [1, 2]])
            # dnb = den + N*BIAS   (Act Copy-with-bias; same table as tanh)
            nc.scalar.activation(
                dnb[:, half * 4 : half * 4 + 4].rearrange("p (c q) -> p c q", q=2),
                den, AF.Copy, bias=DENOM_BIAS,
            )
        rec = temps.tile([128, 8], FP32, tag="rec", bufs=2)
        nc.vector.reciprocal(rec[:], dnb[:])
        for half in range(2):
            t = o_tiles[half][:]
            numer = bass.AP(tensor=t.tensor, offset=t.offset,
                            ap=[t.ap[0], [130, 2], [64, 2], [1, 64]])
            rc = rec[:, half * 4 : half * 4 + 4].rearrange("p (c q) -> p c q", q=2)
            nc.vector.tensor_tensor(
                out_sb[:, half * 256 : half * 256 + 256].rearrange(
                    "p (c q e) -> p c q e", q=2, e=64
                ),
                numer,
                bcast(rc, 64),
                op=ALU.mult,
            )
        nc.sync.dma_start(out_blk[blk], out_sb[:])


def build_core(tc, pools, consts, qk_ap, v_ap, a_ap, w_ap, out_ap):
    nc = tc.nc
    temps, psX, psT, psN, psC, psO, persist = pools

    qk_tiles = [None] * HEADS_PER_CORE
    rs_tiles = [None] * HEADS_PER_CORE
    norm_state = {}

    def norm_step(h, step):
        # step 0: whole-head qk DMA; 1-4: sq+reduce over 2-block chunks;
        # 5: sqrt + recip + fp16 convert
        if step == 0:
            qk_sb = persist.tile(
                [128, NBLK, 8, 64], FP16, tag=f"qk{h}", bufs=1, name=f"qk_sb{h}"
            )
            nc.sync.dma_start(
                qk_sb[:], qk_ap[h].rearrange("(blk p a) c -> p blk a c", p=128, a=8)
            )
            nsq = temps.tile([128, 64], FP32, tag=f"nsq{h}", bufs=1, name=f"nsq{h}")
            qk_tiles[h] = qk_sb
            norm_state[h] = nsq
        elif step <= 4:
            j = step - 1
            qk_sb, nsq = qk_tiles[h], norm_state[h]
            sq = temps.tile([128, 2, 8, 64], FP16, tag="sq", name="sq")
            nc.vector.tensor_mul(
                sq[:], qk_sb[:, 2 * j : 2 * j + 2], qk_sb[:, 2 * j : 2 * j + 2]
            )
            nc.vector.reduce_sum(
                nsq[:, 16 * j : 16 * j + 16],
                sq[:].rearrange("p b a c -> p (b a) c"),
                axis=AX.X,
            )
        else:
            nsq = norm_state[h]
            nrm = temps.tile([128, 64], FP32, tag=f"nrm{h}", bufs=1, name=f"nrm{h}")
            nc.scalar.sqrt(nrm[:], nsq[:])
            rs = temps.tile([128, 64], FP32, tag=f"rs{h}", bufs=1, name=f"rs{h}")
            nc.vector.reciprocal(rs[:], nrm[:])
            rs16 = temps.tile([128, 64], FP16, tag=f"rs16{h}", bufs=1, name=f"rs16{h}")
            nc.scalar.copy(rs16[:], rs[:])
            rs_tiles[h] = rs16

    for s in range(6):
        norm_step(0, s)
    for h in range(HEADS_PER_CORE):
        hook = (
            (lambda step, hh=h + 1: norm_step(hh, step))
            if h + 1 < HEADS_PER_CORE
            else None
        )
        # G = anchor.T @ W_hash for this head
        a_sb = temps.tile([128, 2, 64], FP16, tag="a_sb")
        w_sb = temps.tile([128, 2, 64], FP16, tag="w_sb")
        nc.sync.dma_start(a_sb[:], a_ap[h].rearrange("(t p) c -> p t c", p=128))
        nc.sync.dma_start(w_sb[:], w_ap[h].rearrange("(t p) c -> p t c", p=128))
        g_ps = psT.tile([64, 64], FP32, tag="pT")
        for t in range(2):
            nc.tensor.matmul(
                g_ps[:], a_sb[:, t, :], w_sb[:, t, :], start=(t == 0), stop=(t == 1)
            )
        build_head(
            tc, pools, consts, h, v_ap, out_ap[h], qk_tiles[h], rs_tiles[h], g_ps,
            norm_hook=hook,
        )


def build_bass(repeat=1):
    nc = bacc.Bacc("TRN2", target_bir_lowering=False, debug=False, num_devices=8)
    hp = HEADS_PER_CORE
    qk_ap = nc.dram_tensor("qk", (hp, N, C), FP16, kind="ExternalInput").ap()
    v_ap = nc.dram_tensor("v", (hp, N, C), FP16, kind="ExternalInput").ap()
    a_ap = nc.dram_tensor("anchor", (hp, 256, C), FP16, kind="ExternalInput").ap()
    w_ap = nc.dram_tensor("W_hash", (hp, 256, NBITS), FP16, kind="ExternalInput").ap()
    out_ap = nc.dram_tensor("out", (hp, N, C), FP16, kind="ExternalOutput").ap()

    with tile.TileContext(nc) as tc:
        with ExitStack() as ctx:
            singles = ctx.enter_context(tc.tile_pool(name="singles", bufs=1))
            temps = ctx.enter_context(tc.tile_pool(name="temps", bufs=3))
            persist = ctx.enter_context(tc.tile_pool(name="persist", bufs=1))
            psX = ctx.enter_context(tc.tile_pool(name="psX", bufs=2, space="PSUM"))
            psT = ctx.enter_context(tc.tile_pool(name="psT", bufs=1, space="PSUM"))
            psN = ctx.enter_context(tc.tile_pool(name="psN", bufs=2, space="PSUM"))
            psC = ctx.enter_context(tc.tile_pool(name="psC", bufs=1, space="PSUM"))
            psO = ctx.enter_context(tc.tile_pool(name="psO", bufs=2, space="PSUM"))
            pools = (temps, psX, psT, psN, psC, psO, persist)

            ident = singles.tile([128, 128], FP16)
            make_identity(nc, ident[:])
            ident65 = singles.tile([128, 128], FP16)
            nc.scalar.activation(ident65[:], ident[:], AF.Copy, scale=float(BIAS))
            consts = (ident, ident65)

            if repeat == 1:
                build_core(tc, pools, consts, qk_ap, v_ap, a_ap, w_ap, out_ap)
            else:
                with tc.For_i(0, repeat, 1):
                    build_core(tc, pools, consts, qk_ap, v_ap, a_ap, w_ap, out_ap)
    nc.compile()
    return nc


_NC_CACHE = None
_RUN_KWARGS = {}
_LAST_RESULTS = None


def kernel(qk, v, anchor, W_hash):
    global _NC_CACHE
    if _NC_CACHE is None:
        _NC_CACHE = build_bass()
    nc = _NC_CACHE

    qk = np.ascontiguousarray(qk, dtype=np.float16).reshape(B * H, N, C)
    v = np.ascontiguousarray(v, dtype=np.float16).reshape(B * H, N, C)
    anchor = np.ascontiguousarray(anchor, dtype=np.float16)
    W_hash = np.ascontiguousarray(W_hash, dtype=np.float16)

    in_maps = []
    for core in range(8):
        bh = np.arange(core * HEADS_PER_CORE, (core + 1) * HEADS_PER_CORE)
        h_idx = bh % H
        in_maps.append(
            {
                "qk": qk[bh],
                "v": v[bh],
                "anchor": np.ascontiguousarray(anchor[h_idx]),
                "W_hash": np.ascontiguousarray(W_hash[h_idx]),
            }
        )

    res = run_bass_kernel_spmd(nc, in_maps, core_ids=list(range(8)), **_RUN_KWARGS)
    global _LAST_RESULTS
    _LAST_RESULTS = res
    out = np.concatenate([res.results[c]["out"] for c in range(8)], axis=0)
    return out.reshape(B, H, N, C).astype(np.float32)


# revision 21
# speedup vs baseline: 1.0626x; 1.0163x over previous
"""Liteformer fast attention kernel for Trainium2 (8 NeuronCores), v3.

Math (per (b,h) head, N=8192 tokens, C=K=E=64, m=256 anchors):
    xhat = qk / ||qk||_row
    phi  = tanh((xhat @ anchor.T) @ W_hash) = tanh(xhat @ G),  G = anchor.T @ W_hash  [64,64]
    kcum = phi.sum(axis=0)                                  [64]
    ctx  = phi.T @ v                                        [64,64]
    out  = (phi @ ctx + 65*v) / (phi @ kcum + 8192*65)[:,None]

Sharding: B*H = 32 heads split 4-per-core across 8 cores (fully independent).

v3: pass2 of head h interleaved block-by-block with pass1 of head h+1 so the
pass2 PE->Act(dnb)->DVE(rec) latency chain is hidden behind pass1 work.
All inputs fp16 (host-converted); output fp16, host-cast to fp32.
"""

import sys

sys.path.insert(0, "/opt/trn_rl_repo")

from contextlib import ExitStack

import numpy as np

import concourse.bass as bass
import concourse.tile as tile
from concourse import bacc, mybir
from concourse.bass_utils import run_bass_kernel_spmd
from concourse.masks import make_identity

B, H, N, C = 2, 16, 8192, 64
NBITS = 64
BIAS = NBITS + 1  # 65
DENOM_BIAS = float(N) * BIAS  # 532480
HEADS_PER_CORE = (B * H) // 8  # 4
NBLK = N // 1024  # 8 blocks of 1024 tokens per head
FP32 = mybir.dt.float32
FP16 = mybir.dt.float16
AX = mybir.AxisListType
AF = mybir.ActivationFunctionType
ALU = mybir.AluOpType


def bcast(ap, n):
    """Append a zero-stride broadcast dim of size n to an AP."""
    return bass.AP(tensor=ap.tensor, offset=ap.offset, ap=ap.ap + [[0, n]])


class CoreBuilder:
    def __init__(self, tc, pools, consts, qk_ap, v_ap, a_ap, w_ap, out_ap):
        self.tc = tc
        self.nc = tc.nc
        (self.temps, self.psX, self.psT, self.psN, self.psC, self.psO,
         self.persist) = pools
        self.ident, self.ident65 = consts
        self.qk_ap, self.v_ap, self.a_ap, self.w_ap = qk_ap, v_ap, a_ap, w_ap
        self.out_ap = out_ap
        self.qk_tiles = [None] * HEADS_PER_CORE
        self.rs_tiles = [None] * HEADS_PER_CORE
        self.nsq_tiles = [None] * HEADS_PER_CORE

    # ---------------- norm phase (next head), emitted in small steps -------
    def norm_step(self, h, step):
        nc, temps = self.nc, self.temps
        if step == 0:
            qk_sb = self.persist.tile(
                [128, NBLK, 8, 64], FP16, tag=f"qk{h}", bufs=1, name=f"qk_sb{h}"
            )
            nc.sync.dma_start(
                qk_sb[:],
                self.qk_ap[h].rearrange("(blk p a) c -> p blk a c", p=128, a=8),
            )
            nsq = temps.tile([128, 64], FP16, tag=f"nsq{h}", bufs=1, name=f"nsq{h}")
            self.qk_tiles[h] = qk_sb
            self.nsq_tiles[h] = nsq
        elif step <= 4:
            j = step - 1
            qk_sb, nsq = self.qk_tiles[h], self.nsq_tiles[h]
            sq = temps.tile([128, 2, 8, 64], FP16, tag="sq", name="sq")
            nc.vector.tensor_mul(
                sq[:], qk_sb[:, 2 * j : 2 * j + 2], qk_sb[:, 2 * j : 2 * j + 2]
            )
            with nc.allow_low_precision("rownorm sums fit fp16 (~64 +- 40)"):
                nc.vector.reduce_sum(
                    nsq[:, 16 * j : 16 * j + 16],
                    sq[:].rearrange("p b a c -> p (b a) c"),
                    axis=AX.X,
                )
        else:
            nsq = self.nsq_tiles[h]
            nrm = temps.tile([128, 64], FP32, tag=f"nrm{h}", bufs=1, name=f"nrm{h}")
            nc.scalar.sqrt(nrm[:], nsq[:])
            rs = temps.tile([128, 64], FP32, tag=f"rs{h}", bufs=1, name=f"rs{h}")
            nc.vector.reciprocal(rs[:], nrm[:])
            rs16 = temps.tile([128, 64], FP16, tag=f"rs16{h}", bufs=1, name=f"rs16{h}")
            nc.scalar.copy(rs16[:], rs[:])
            self.rs_tiles[h] = rs16

    # ---------------- pass 1 (one head), with interleave hooks -------------
    def pass1_head(self, h, pass2_hook):
        nc, temps = self.nc, self.temps
        nxt = h + 1 if h + 1 < HEADS_PER_CORE else None

        # G = anchor.T @ W_hash
        a_sb = temps.tile([128, 2, 64], FP16, tag="a_sb")
        w_sb = temps.tile([128, 2, 64], FP16, tag="w_sb")
        nc.sync.dma_start(a_sb[:], self.a_ap[h].rearrange("(t p) c -> p t c", p=128))
        nc.sync.dma_start(w_sb[:], self.w_ap[h].rearrange("(t p) c -> p t c", p=128))
        g_ps = self.psT.tile([64, 64], FP32, tag="pT")
        for t in range(2):
            nc.tensor.matmul(
                g_ps[:], a_sb[:, t, :], w_sb[:, t, :], start=(t == 0), stop=(t == 1)
            )
        # g2 = block-diag [G 0; 0 G] fp16
        g2 = temps.tile([128, 128], FP16, tag="g2", bufs=2)
        nc.gpsimd.memset(g2[:], 0.0)
        nc.scalar.copy(g2[0:64, 0:64], g_ps[:])
        nc.scalar.copy(g2[64:128, 64:128], g_ps[:])

        vsb = self.persist.tile([128, NBLK, 8, 64], FP16, tag="vsb", bufs=2)
        nc.sync.dma_start(
            vsb[:], self.v_ap[h].rearrange("(blk p a) c -> p blk a c", p=128, a=8)
        )
        phiT = self.persist.tile([128, NBLK, 512], FP16, tag="phiT", bufs=2)
        kcacc = temps.tile([128, NBLK], FP32, tag="kcacc", bufs=2)
        ctx_ps = self.psC.tile([64, 64], FP32, tag="ctx")
        qk_sb, rs = self.qk_tiles[h], self.rs_tiles[h]

        if nxt is not None:
            self.norm_step(nxt, 0)

        for blk in range(NBLK):
            if nxt is not None and 2 <= blk <= 6:
                self.norm_step(nxt, blk - 1)

            xn = temps.tile([128, 8, 64], FP16, tag="xn")
            nc.vector.tensor_mul(
                xn[:],
                qk_sb[:, blk],
                bcast(
                    rs[:, blk * 8 : (blk + 1) * 8].rearrange("p (a o) -> p a o", o=1),
                    64,
                ),
            )
            xt_ps = self.psX.tile([128, 512], FP16, tag="xt_ps")
            xn2 = xn[:].rearrange("p a c -> p (a c)")
            for j in range(4):
                nc.tensor.transpose(
                    xt_ps[:, j * 128 : (j + 1) * 128],
                    xn2[:, j * 128 : (j + 1) * 128],
                    self.ident[:],
                )
            xt = temps.tile([128, 512], FP16, tag="xt")
            nc.vector.tensor_copy(xt[:], xt_ps[:])

            pT_ps = self.psT.tile([128, 512], FP32, tag="pT")
            nc.tensor.matmul(pT_ps[:], g2[:], xt[:], start=True, stop=True)
            nc.scalar.activation(
                phiT[:, blk, :], pT_ps[:], AF.Tanh, accum_out=kcacc[:, blk : blk + 1]
            )

            pN_ps = self.psN.tile([128, 512], FP32, tag="pN")
            for j in range(4):
                nc.tensor.matmul(
                    pN_ps[:, j * 128 : (j + 1) * 128],
                    xt[:, j * 128 : (j + 1) * 128],
                    g2[:],
                    start=True,
                    stop=True,
                )
            phi = temps.tile([128, 8, 64], FP16, tag="phi")
            nc.scalar.activation(phi[:].rearrange("p a c -> p (a c)"), pN_ps[:], AF.Tanh)

            for a in range(8):
                nc.tensor.matmul(
                    ctx_ps[:],
                    phi[:, a, :],
                    vsb[:, blk, a, :],
                    start=(blk == 0 and a == 0),
                    stop=(blk == NBLK - 1 and a == 7),
                )

            if pass2_hook is not None:
                pass2_hook(blk)

        # ---- head tail: kcum fold + cc_n build ---------------------------
        kcr = temps.tile([128, 1], FP32, tag="kcr")
        nc.vector.reduce_sum(kcr[:], kcacc[:], axis=AX.X)
        kup = temps.tile([64, 1], FP32, tag="kup")
        nc.scalar.copy(kup[:], kcr[64:128, :])
        kcs = temps.tile([64, 1], FP32, tag="kcs")
        nc.vector.tensor_tensor(kcs[:], kcr[0:64, :], kup[:], op=ALU.add)

        cc_n = temps.tile([128, 130], FP16, tag="cc_n", bufs=2)
        nc.gpsimd.memset(cc_n[:], 0.0)
        nc.scalar.copy(cc_n[0:64, 0:64], ctx_ps[:])
        nc.scalar.copy(cc_n[64:128, 64:128], ctx_ps[:])
        nc.scalar.copy(cc_n[0:64, 128:129], kcs[:])
        nc.scalar.copy(cc_n[64:128, 129:130], kcs[:])
        return dict(h=h, phiT=phiT, vsb=vsb, cc_n=cc_n)

    # ---------------- pass 2 (one block of one head) -----------------------
    def pass2_block(self, st, blk):
        nc, temps = self.nc, self.temps
        phiT, vsb, cc_n = st["phiT"], st["vsb"], st["cc_n"]
        out_blk = self.out_ap[st["h"]].rearrange(
            "(blk p a) c -> blk p (a c)", p=128, a=8
        )
        out_sb = temps.tile([128, 512], FP16, tag="out_sb", bufs=2)
        dnb = temps.tile([128, 8], FP32, tag="dnb", bufs=2)
        o_tiles = []
        for half in range(2):
            o = self.psO.tile([128, 260], FP32, tag="o", bufs=2)
            o_tiles.append(o)
            for q in range(2):
                ch = half * 2 + q
                nc.tensor.matmul(
                    o[:, q * 130 : q * 130 + 130],
                    phiT[:, blk, ch * 128 : (ch + 1) * 128],
                    cc_n[:],
                    start=True,
                    stop=False,
                    skip_group_check=True,
                )
                nc.tensor.matmul(
                    o[:, q * 130 : q * 130 + 128],
                    self.ident65[:],
                    vsb[:, blk, ch * 2 : ch * 2 + 2, :],
                    start=False,
                    stop=True,
                    skip_group_check=True,
                )
            t = o[:]
            den = bass.AP(tensor=t.tensor, offset=t.offset + 128,
                          ap=[t.ap[0], [130, 2], [1, 2]])
            nc.scalar.activation(
                dnb[:, half * 4 : half * 4 + 4].rearrange("p (c q) -> p c q", q=2),
                den, AF.Copy, bias=DENOM_BIAS,
            )
        rec = temps.tile([128, 8], FP32, tag="rec", bufs=2)
        nc.vector.reciprocal(rec[:], dnb[:])
        for half in range(2):
            t = o_tiles[half][:]
            numer = bass.AP(tensor=t.tensor, offset=t.offset,
                            ap=[t.ap[0], [130, 2], [64, 2], [1, 64]])
            rc = rec[:, half * 4 : half * 4 + 4].rearrange("p (c q) -> p c q", q=2)
            nc.vector.tensor_tensor(
                out_sb[:, half * 256 : half * 256 + 256].rearrange(
                    "p (c q e) -> p c q e", q=2, e=64
                ),
                numer,
                bcast(rc, 64),
                op=ALU.mult,
            )
        nc.sync.dma_start(out_blk[blk], out_sb[:])

    # ---------------- whole core ------------------------------------------
    def build(self):
        for s in range(6):
            self.norm_step(0, s)
        st = None
        for h in range(HEADS_PER_CORE):
            hook = (lambda blk, s=st: self.pass2_block(s, blk)) if st else None
            st = self.pass1_head(h, hook)
        for blk in range(NBLK):
            self.pass2_block(st, blk)


def build_bass(repeat=1):
    nc = bacc.Bacc("TRN2", target_bir_lowering=False, debug=False, num_devices=8)
    hp = HEADS_PER_CORE
    qk_ap = nc.dram_tensor("qk", (hp, N, C), FP16, kind="ExternalInput").ap()
    v_ap = nc.dram_tensor("v", (hp, N, C), FP16, kind="ExternalInput").ap()
    a_ap = nc.dram_tensor("anchor", (hp, 256, C), FP16, kind="ExternalInput").ap()
    w_ap = nc.dram_tensor("W_hash", (hp, 256, NBITS), FP16, kind="ExternalInput").ap()
    out_ap = nc.dram_tensor("out", (hp, N, C), FP16, kind="ExternalOutput").ap()

    with tile.TileContext(nc) as tc:
        with ExitStack() as ctx:
            singles = ctx.enter_context(tc.tile_pool(name="singles", bufs=1))
            temps = ctx.enter_context(tc.tile_pool(name="temps", bufs=3))
            persist = ctx.enter_context(tc.tile_pool(name="persist", bufs=1))
            psX = ctx.enter_context(tc.tile_pool(name="psX", bufs=2, space="PSUM"))
            psT = ctx.enter_context(tc.tile_pool(name="psT", bufs=1, space="PSUM"))
            psN = ctx.enter_context(tc.tile_pool(name="psN", bufs=2, space="PSUM"))
            psC = ctx.enter_context(tc.tile_pool(name="psC", bufs=1, space="PSUM"))
            psO = ctx.enter_context(tc.tile_pool(name="psO", bufs=2, space="PSUM"))
            pools = (temps, psX, psT, psN, psC, psO, persist)

            ident = singles.tile([128, 128], FP16)
            make_identity(nc, ident[:])
            ident65 = singles.tile([128, 128], FP16)
            nc.scalar.activation(ident65[:], ident[:], AF.Copy, scale=float(BIAS))
            consts = (ident, ident65)

            builder = CoreBuilder(tc, pools, consts, qk_ap, v_ap, a_ap, w_ap, out_ap)
            if repeat == 1:
                builder.build()
            else:
                with tc.For_i(0, repeat, 1):
                    builder.build()
    nc.compile()
    return nc


_NC_CACHE = None
_RUN_KWARGS = {}
_LAST_RESULTS = None


def kernel(qk, v, anchor, W_hash):
    global _NC_CACHE
    if _NC_CACHE is None:
        _NC_CACHE = build_bass()
    nc = _NC_CACHE

    qk = np.ascontiguousarray(qk, dtype=np.float16).reshape(B * H, N, C)
    v = np.ascontiguousarray(v, dtype=np.float16).reshape(B * H, N, C)
    anchor = np.ascontiguousarray(anchor, dtype=np.float16)
    W_hash = np.ascontiguousarray(W_hash, dtype=np.float16)

    in_maps = []
    for core in range(8):
        bh = np.arange(core * HEADS_PER_CORE, (core + 1) * HEADS_PER_CORE)
        h_idx = bh % H
        in_maps.append(
            {
                "qk": qk[bh],
                "v": v[bh],
                "anchor": np.ascontiguousarray(anchor[h_idx]),
                "W_hash": np.ascontiguousarray(W_hash[h_idx]),
            }
        )

    res = run_bass_kernel_spmd(nc, in_maps, core_ids=list(range(8)), **_RUN_KWARGS)
    global _LAST_RESULTS
    _LAST_RESULTS = res
    out = np.concatenate([res.results[c]["out"] for c in range(8)], axis=0)
    return out.reshape(B, H, N, C).astype(np.float32)
